# revision 11
# baseline (speedup 1.0000x reference)
"""Windowed multi-head attention (Swin-style) Bass kernel for Trainium2.

Full inputs -> shard over 8 NeuronCores (data-parallel over windows) -> full output.

Math per window w (n=60 tokens, d=256, h=8 heads, dh=32):
  qkv = x_w @ w_qkv ; sim = scale*q_h @ k_h^T + bias_h ; attn = softmax(sim)
  out_w = concat_h(attn @ v_h) @ w_out

Device formulation (per group of 2 windows, 120 token rows):
  - host pre-transposes x -> xT [256, 120] (bf16), pre-scales w_q by dh^-0.5
  - qT, kT computed head-dim-major via matmul(lhsT=w_slice, rhs=xT)
  - v computed token-major via matmul(lhsT=xT, rhs=w_v)
  - simT[j,i] = matmul(lhsT=kT_h, rhs=qT_h)   (j on partitions)
  - eT = exp(simT) * expbias  (expbias = exp(bias^T) host-precomputed, 0 in
    cross-window blocks -> masks the 2-window batching for free)
  - av: matmul(lhsT=eT, rhs=[v_h | ones]) -> [120, 33]: cols 0-31 unnormalized
    attn@v (token-major!), col 32 = softmax denominator s_i (per-partition)
  - normalize: ao[:, h*32:+32] = av[:, :32] * (1/s)   (per-partition scalar)
  - PE-transpose ao -> aoT, proj: matmul(lhsT=aoT, rhs=w_out) -> [120, 256]
"""

import os
from contextlib import ExitStack

import numpy as np
import ml_dtypes

N_CORES = 8
WH, WW = 6, 10
N_TOK = WH * WW          # 60 tokens per window
DIM = 256
HEADS = 8
DH = 32
B_WINDOWS = 16 * 16 * 16  # 4096
GROUPS_TOTAL = B_WINDOWS // 2        # 2048 groups of 2 windows
NGROUPS = GROUPS_TOTAL // N_CORES    # 256 per core
GT = 2 * N_TOK           # 120 rows per group

BF16 = ml_dtypes.bfloat16

LAST_RESULT = None  # BassKernelResults from the most recent run (for test.py)

_CACHE = {}


def _build_bass(ngroups: int):
    import concourse.bass as bass
    import concourse.tile as tile
    from concourse import bacc, mybir
    from concourse.masks import make_identity

    fp32 = mybir.dt.float32
    bf16 = mybir.dt.bfloat16

    nc = bacc.Bacc("TRN2", debug=False, enable_asserts=False)

    xt_d = nc.dram_tensor("xt", [ngroups, 2, 128, GT], bf16, kind="ExternalInput").ap()
    wqkv_d = nc.dram_tensor("wqkv", [2, 128, 768], bf16, kind="ExternalInput").ap()
    wout_d = nc.dram_tensor("wout", [2, 128, 256], bf16, kind="ExternalInput").ap()
    expb_d = nc.dram_tensor("expb", [GT, HEADS, GT], bf16, kind="ExternalInput").ap()
    out_d = nc.dram_tensor("out", [ngroups, GT, 256], fp32, kind="ExternalOutput").ap()

    with tile.TileContext(nc) as tc, ExitStack() as ctx:
        consts = ctx.enter_context(tc.tile_pool(name="consts", bufs=1))

        wqkv_sb = consts.tile([128, 2, 768], bf16)
        for kh in range(2):
            nc.gpsimd.dma_start(out=wqkv_sb[:, kh, :], in_=wqkv_d[kh])
        wout_sb = consts.tile([128, 2, 256], bf16)
        for kh in range(2):
            nc.gpsimd.dma_start(out=wout_sb[:, kh, :], in_=wout_d[kh])
        expb_sb = consts.tile([GT, HEADS, GT], bf16)
        nc.gpsimd.dma_start(out=expb_sb, in_=expb_d)
        ident = consts.tile([128, 128], bf16)
        make_identity(nc, ident)

        xpool = ctx.enter_context(tc.tile_pool(name="xp", bufs=ngroups))
        qkpool = ctx.enter_context(tc.tile_pool(name="qk", bufs=2))
        etpool = ctx.enter_context(tc.tile_pool(name="et", bufs=3))
        vpool = ctx.enter_context(tc.tile_pool(name="vp", bufs=2))
        aopool = ctx.enter_context(tc.tile_pool(name="ao", bufs=2))
        aotpool = ctx.enter_context(tc.tile_pool(name="aot", bufs=2))
        rpool = ctx.enter_context(tc.tile_pool(name="rp", bufs=2))
        opool = ctx.enter_context(tc.tile_pool(name="op", bufs=3))

        pqk = ctx.enter_context(tc.tile_pool(name="pqk", bufs=2, space="PSUM"))
        pst = ctx.enter_context(tc.tile_pool(name="pst", bufs=2, space="PSUM"))
        pav = ctx.enter_context(tc.tile_pool(name="pav", bufs=2, space="PSUM"))
        pmisc = ctx.enter_context(tc.tile_pool(name="pmisc", bufs=2, space="PSUM"))

        for g in range(ngroups):
            xt = xpool.tile([128, 2, GT], bf16)
            nc.sync.dma_start(
                out=xt, in_=xt_d[g].rearrange("k p t -> p k t")
            )

            # qT / kT: chunks 0,1 = q (features 0..255), 2,3 = k
            qkT = qkpool.tile([128, 4, GT], bf16)
            for c in range(4):
                ps = pqk.tile([128, GT], fp32)
                for kh in range(2):
                    nc.tensor.matmul(
                        ps,
                        lhsT=wqkv_sb[:, kh, c * 128:(c + 1) * 128],
                        rhs=xt[:, kh, :],
                        start=(kh == 0),
                        stop=(kh == 1),
                    )
                if c % 2 == 0:
                    nc.vector.tensor_copy(out=qkT[:, c, :], in_=ps)
                else:
                    nc.scalar.activation(
                        out=qkT[:, c, :], in_=ps,
                        func=mybir.ActivationFunctionType.Copy,
                    )

            # v token-major, with interleaved ones column: v1[:, h, 0:32]=v_h, [:,h,32]=1
            pv = pmisc.tile([120, 256], fp32, tag="m")
            for kh in range(2):
                nc.tensor.matmul(
                    pv,
                    lhsT=xt[:, kh, :],
                    rhs=wqkv_sb[:, kh, 512:768],
                    start=(kh == 0),
                    stop=(kh == 1),
                )
            v1 = vpool.tile([GT, HEADS, DH + 1], bf16)
            nc.gpsimd.memset(v1[:, :, DH:DH + 1], 1.0)
            nc.vector.tensor_copy(
                out=v1[:, :, 0:DH],
                in_=pv.rearrange("p (h d) -> p h d", h=HEADS),
            )

            # attention per head
            pav_t = pav.tile([GT, HEADS, DH + 1], fp32)
            for h in range(8):
                chunk, poff = h // 4, (h % 4) * DH
                kT_h = qkT[poff:poff + DH, 2 + chunk, :]
                qT_h = qkT[poff:poff + DH, 0 + chunk, :]
                ps_sim = pst.tile([GT, GT], fp32)
                nc.tensor.matmul(
                    ps_sim, lhsT=kT_h, rhs=qT_h, start=True, stop=True,
                    tile_position=(poff, 0),
                )
                et = etpool.tile([GT, GT], bf16)
                nc.scalar.activation(
                    out=et, in_=ps_sim, func=mybir.ActivationFunctionType.Exp
                )
                nc.vector.tensor_mul(et, et, expb_sb[:, h, :])
                nc.tensor.matmul(
                    pav_t[:, h, :], lhsT=et, rhs=v1[:, h, :], start=True, stop=True
                )

            # normalize: r = 1/s, ao = av * r
            r = rpool.tile([GT, HEADS, 1], fp32)
            nc.vector.reciprocal(out=r, in_=pav_t[:, :, DH:DH + 1])
            ao = aopool.tile([GT, 2, 128], bf16)
            aov = ao.rearrange("p a (b d) -> p (a b) d", d=DH)
            for h in range(8):
                nc.vector.tensor_scalar_mul(
                    aov[:, h, :], pav_t[:, h, 0:DH], r[:, h, :]
                )

            # transpose ao -> aoT
            aoT = aotpool.tile([128, 2, GT], bf16)
            for kh in range(2):
                pt = pmisc.tile([128, GT], bf16, tag="m")
                nc.tensor.transpose(pt, ao[:, kh, :], ident[0:GT, 0:GT])
                nc.scalar.activation(
                    out=aoT[:, kh, :], in_=pt,
                    func=mybir.ActivationFunctionType.Copy,
                )

            # out projection
            po = pmisc.tile([120, 256], fp32, tag="m")
            for kh in range(2):
                nc.tensor.matmul(
                    po,
                    lhsT=aoT[:, kh, :],
                    rhs=wout_sb[:, kh, :],
                    start=(kh == 0),
                    stop=(kh == 1),
                )
            osb = opool.tile([GT, 256], fp32)
            nc.scalar.activation(
                out=osb, in_=po, func=mybir.ActivationFunctionType.Copy
            )
            nc.scalar.dma_start(out=out_d[g], in_=osb)

    nc.compile()
    return nc


def _get_bass(ngroups: int):
    if ngroups not in _CACHE:
        _CACHE[ngroups] = _build_bass(ngroups)
    return _CACHE[ngroups]


def _host_prep(x, w_qkv, w_out, bias_table, rel_pos_indices):
    x = np.asarray(x, dtype=np.float32)
    w_qkv = np.asarray(w_qkv, dtype=np.float32)
    w_out = np.asarray(w_out, dtype=np.float32)
    bias_table = np.asarray(bias_table, dtype=np.float32)
    rel_pos_indices = np.asarray(rel_pos_indices)

    xg = x.reshape(GROUPS_TOTAL, GT, DIM)
    xT = np.ascontiguousarray(xg.transpose(0, 2, 1)).astype(BF16)
    xT = xT.reshape(GROUPS_TOTAL, 2, 128, GT)

    wq = w_qkv.copy()
    wq[:, :DIM] *= DH ** -0.5
    wqkv_h = np.ascontiguousarray(wq.reshape(2, 128, 768)).astype(BF16)
    wout_h = np.ascontiguousarray(w_out.reshape(2, 128, 256)).astype(BF16)

    bias = bias_table[rel_pos_indices]        # [60, 60, H]  (i, j, h)
    eb = np.exp(bias).transpose(1, 2, 0)      # [j, h, i]
    expb = np.zeros((GT, HEADS, GT), dtype=np.float32)
    for blk in range(2):
        expb[blk * N_TOK:(blk + 1) * N_TOK, :, blk * N_TOK:(blk + 1) * N_TOK] = eb
    expb_h = expb.astype(BF16)
    return xT, wqkv_h, wout_h, expb_h


def kernel(x, w_qkv, w_out, bias_table, rel_pos_indices):
    global LAST_RESULT
    from concourse.bass_utils import run_bass_kernel_spmd

    xT, wqkv_h, wout_h, expb_h = _host_prep(
        x, w_qkv, w_out, bias_table, rel_pos_indices
    )

    nc = _get_bass(NGROUPS)
    in_maps = []
    for c in range(N_CORES):
        in_maps.append({
            "xt": np.ascontiguousarray(xT[c * NGROUPS:(c + 1) * NGROUPS]),
            "wqkv": wqkv_h,
            "wout": wout_h,
            "expb": expb_h,
        })

    res = run_bass_kernel_spmd(
        nc, in_maps, core_ids=list(range(N_CORES)),
        trace=bool(int(os.environ.get("KERNEL_TRACE", "0"))),
    )
    LAST_RESULT = res

    out = np.concatenate([res.results[c]["out"] for c in range(N_CORES)], axis=0)
    out = out.reshape(16, 16, 16, WH, WW, DIM).astype(np.float32)
    return out


# revision 16
# speedup vs baseline: 1.2954x; 1.2954x over previous
"""Windowed multi-head attention (Swin-style) Bass kernel for Trainium2.

Full inputs -> shard over 8 NeuronCores (data-parallel over windows) -> full output.

Math per window w (n=60 tokens, d=256, h=8 heads, dh=32):
  qkv = x_w @ w_qkv ; sim = scale*q_h @ k_h^T + bias_h ; attn = softmax(sim)
  out_w = concat_h(attn @ v_h) @ w_out

Device formulation (per group of 2 windows, 120 token rows):
  - host pre-transposes x -> xT [256, 120] (bf16), pre-scales w_q by dh^-0.5
  - qT, kT head-dim-major via matmul(lhsT=w_slice, rhs=xT), batched over
    group PAIRS (N=240 streams)
  - simT for 4 heads in ONE matmul: lhsT = kT_chunk [128,120] dense,
    rhs = qT block-diagonal [128, 4*120] -> psum [120 (j), 480 (h,i)]
  - eT = exp(simT) * expbias  (expbias = exp(bias^T), 0 in cross-window
    blocks -> masks the 2-window batching AND the 4-head batching for free)
  - av per head: matmul(lhsT=eT_h, rhs=[v_h | ones]) -> [120, 33]: cols 0-31
    unnormalized attn@v token-major, col 32 = softmax denominator s_i
  - normalize: ao[:, h*32:+32] = av[:, :32] * (1/s)  (per-partition scalar)
  - PE-transpose ao -> aoT, proj: matmul(lhsT=aoT, rhs=w_out) -> [120, 256]
"""

import os
from contextlib import ExitStack

import numpy as np
import ml_dtypes

os.environ.setdefault("JAX_COMPILATION_CACHE_DIR", "/tmp/jaxcache")
os.environ.setdefault("JAX_PERSISTENT_CACHE_MIN_COMPILE_TIME_SECS", "2")

N_CORES = 8
WH, WW = 6, 10
N_TOK = WH * WW          # 60 tokens per window
DIM = 256
HEADS = 8
DH = 32
B_WINDOWS = 16 * 16 * 16  # 4096
GROUPS_TOTAL = B_WINDOWS // 2        # 2048 groups of 2 windows
NGROUPS = GROUPS_TOTAL // N_CORES    # 256 per core
GT = 2 * N_TOK           # 120 rows per group

BF16 = ml_dtypes.bfloat16

LAST_RESULT = None  # BassKernelResults from the most recent run (for test.py)

_CACHE = {}


def _build_bass(ngroups: int):
    import concourse.bass as bass
    import concourse.tile as tile
    from concourse import bacc, mybir
    from concourse.masks import make_identity

    fp32 = mybir.dt.float32
    bf16 = mybir.dt.bfloat16

    nc = bacc.Bacc("TRN2", debug=False, enable_asserts=False)

    xt_d = nc.dram_tensor("xt", [ngroups, 2, 128, GT], bf16, kind="ExternalInput").ap()
    wqkv_d = nc.dram_tensor("wqkv", [2, 128, 768], bf16, kind="ExternalInput").ap()
    wout_d = nc.dram_tensor("wout", [2, 128, 256], bf16, kind="ExternalInput").ap()
    expb_d = nc.dram_tensor("expb", [GT, HEADS, GT], bf16, kind="ExternalInput").ap()
    out_d = nc.dram_tensor("out", [ngroups, GT, 256], fp32, kind="ExternalOutput").ap()

    npairs = ngroups // 2

    with tile.TileContext(nc) as tc, ExitStack() as ctx:
        consts = ctx.enter_context(tc.tile_pool(name="consts", bufs=1))

        wqkv_sb = consts.tile([128, 2, 768], bf16)
        for kh in range(2):
            nc.gpsimd.dma_start(out=wqkv_sb[:, kh, :], in_=wqkv_d[kh])
        wout_sb = consts.tile([128, 2, 256], bf16)
        for kh in range(2):
            nc.gpsimd.dma_start(out=wout_sb[:, kh, :], in_=wout_d[kh])
        expb_sb = consts.tile([GT, HEADS, GT], bf16)
        nc.gpsimd.dma_start(out=expb_sb, in_=expb_d)
        ident = consts.tile([128, 128], bf16)
        make_identity(nc, ident)

        # persistent block-diagonal qT staging tiles [chunk][group-in-pair];
        # zeroed once, only the diagonal blocks are rewritten each pair
        qbd = [
            [consts.tile([128, 4, GT], bf16, name=f"qbd{c}{gi}", tag=f"qbd{c}{gi}") for gi in range(2)]
            for c in range(2)
        ]
        for c in range(2):
            for gi in range(2):
                nc.vector.memset(qbd[c][gi], 0.0)

        xpool = ctx.enter_context(tc.tile_pool(name="xp", bufs=ngroups // 2))
        ktpool = ctx.enter_context(tc.tile_pool(name="kt", bufs=4))
        etpool = ctx.enter_context(tc.tile_pool(name="et", bufs=4))
        vpool = ctx.enter_context(tc.tile_pool(name="vp", bufs=3))
        aopool = ctx.enter_context(tc.tile_pool(name="ao", bufs=2))
        aotpool = ctx.enter_context(tc.tile_pool(name="aot", bufs=2))
        rpool = ctx.enter_context(tc.tile_pool(name="rp", bufs=2))
        opool = ctx.enter_context(tc.tile_pool(name="op", bufs=3))

        pqk = ctx.enter_context(tc.tile_pool(name="pqk", bufs=2, space="PSUM"))
        pst = ctx.enter_context(tc.tile_pool(name="pst", bufs=2, space="PSUM"))
        pav = ctx.enter_context(tc.tile_pool(name="pav", bufs=2, space="PSUM"))
        pmisc = ctx.enter_context(tc.tile_pool(name="pmisc", bufs=2, space="PSUM"))

        for gp in range(npairs):
            xt = xpool.tile([128, 2, 2, GT], bf16)  # [p, kh, gi, t]
            for kh in range(2):
                nc.sync.dma_start(
                    out=xt[:, kh, :, :],
                    in_=xt_d[2 * gp:2 * gp + 2, kh].rearrange("g p t -> p g t"),
                )

            # fat qkT matmuls over the pair: chunks 0,1 = q ; 2,3 = k
            kt = [
            ktpool.tile([128, 2, GT], bf16, name=f"kt{i}", tag=f"kt{i}") for i in range(2)
        ]
            for c in range(4):
                ps = pqk.tile([128, 2, GT], fp32, tag="ps")
                for kh in range(2):
                    nc.tensor.matmul(
                        ps.rearrange("p a b -> p (a b)"),
                        lhsT=wqkv_sb[:, kh, c * 128:(c + 1) * 128],
                        rhs=xt[:, kh, :, :].rearrange("p a b -> p (a b)"),
                        start=(kh == 0),
                        stop=(kh == 1),
                    )
                if c < 2:
                    # q chunk: scatter diagonal blocks into qbd[c][gi]
                    for gi in range(2):
                        eng = nc.vector if gi == 0 else nc.scalar
                        for m in range(4):
                            blk_out = qbd[c][gi][m * DH:(m + 1) * DH, m, :]
                            blk_in = ps[m * DH:(m + 1) * DH, gi, :]
                            if gi == 0:
                                nc.vector.tensor_copy(out=blk_out, in_=blk_in)
                            else:
                                nc.scalar.activation(
                                    out=blk_out, in_=blk_in,
                                    func=mybir.ActivationFunctionType.Copy,
                                )
                else:
                    for gi in range(2):
                        if gi == 0:
                            nc.vector.tensor_copy(
                                out=kt[gi][:, c - 2, :], in_=ps[:, gi, :]
                            )
                        else:
                            nc.scalar.activation(
                                out=kt[gi][:, c - 2, :], in_=ps[:, gi, :],
                                func=mybir.ActivationFunctionType.Copy,
                            )

            for gi in range(2):
                g = 2 * gp + gi
                # v token-major with interleaved ones col: [120, h, 33]
                pv = pmisc.tile([GT, 256], fp32, tag="m")
                for kh in range(2):
                    nc.tensor.matmul(
                        pv,
                        lhsT=xt[:, kh, gi, :],
                        rhs=wqkv_sb[:, kh, 512:768],
                        start=(kh == 0),
                        stop=(kh == 1),
                    )
                v1 = vpool.tile([GT, HEADS, DH + 1], bf16)
                nc.gpsimd.memset(v1[:, :, DH:DH + 1], 1.0)
                nc.vector.tensor_copy(
                    out=v1[:, :, 0:DH],
                    in_=pv.rearrange("p (h d) -> p h d", h=HEADS),
                )

                # fat simT per chunk: [120 (j), 4*120 (h,i)]
                ets = []
                for c in range(2):
                    ps_sim = pst.tile([GT, 4, GT], fp32, tag="st")
                    nc.tensor.matmul(
                        ps_sim.rearrange("p a b -> p (a b)"),
                        lhsT=kt[gi][:, c, :],
                        rhs=qbd[c][gi].rearrange("p a b -> p (a b)"),
                        start=True, stop=True,
                    )
                    et = etpool.tile([GT, 4, GT], bf16, tag="et")
                    nc.scalar.activation(
                        out=et, in_=ps_sim,
                        func=mybir.ActivationFunctionType.Exp,
                    )
                    nc.gpsimd.tensor_mul(et, et, expb_sb[:, 4 * c:4 * c + 4, :])
                    ets.append(et)

                # av: 8 small matmuls, all inputs ready -> pipeline on PE
                pav_t = pav.tile([GT, HEADS, DH + 1], fp32, tag="av")
                for h in range(8):
                    nc.tensor.matmul(
                        pav_t[:, h, :],
                        lhsT=ets[h // 4][:, h % 4, :],
                        rhs=v1[:, h, :],
                        start=True, stop=True,
                    )

                # normalize: r = 1/s ; ao = av * r
                r = rpool.tile([GT, HEADS, 1], fp32)
                nc.vector.reciprocal(out=r, in_=pav_t[:, :, DH:DH + 1])
                ao = aopool.tile([GT, 2, 128], bf16)
                aov = ao.rearrange("p a (b d) -> p (a b) d", d=DH)
                for h in range(8):
                    nc.vector.tensor_scalar_mul(
                        aov[:, h, :], pav_t[:, h, 0:DH], r[:, h, :]
                    )

                # transpose ao -> aoT
                aoT = aotpool.tile([128, 2, GT], bf16)
                for kh in range(2):
                    pt = pmisc.tile([128, GT], bf16, tag="m")
                    nc.tensor.transpose(pt, ao[:, kh, :], ident[0:GT, 0:GT])
                    nc.scalar.activation(
                        out=aoT[:, kh, :], in_=pt,
                        func=mybir.ActivationFunctionType.Copy,
                    )

                # out projection
                po = pmisc.tile([GT, 256], fp32, tag="m")
                for kh in range(2):
                    nc.tensor.matmul(
                        po,
                        lhsT=aoT[:, kh, :],
                        rhs=wout_sb[:, kh, :],
                        start=(kh == 0),
                        stop=(kh == 1),
                    )
                osb = opool.tile([GT, 256], fp32)
                nc.scalar.activation(
                    out=osb, in_=po, func=mybir.ActivationFunctionType.Copy
                )
                nc.scalar.dma_start(out=out_d[g], in_=osb)

    nc.compile()
    return nc


def _get_bass(ngroups: int):
    if ngroups not in _CACHE:
        _CACHE[ngroups] = _build_bass(ngroups)
    return _CACHE[ngroups]


def _host_prep(x, w_qkv, w_out, bias_table, rel_pos_indices):
    x = np.asarray(x, dtype=np.float32)
    w_qkv = np.asarray(w_qkv, dtype=np.float32)
    w_out = np.asarray(w_out, dtype=np.float32)
    bias_table = np.asarray(bias_table, dtype=np.float32)
    rel_pos_indices = np.asarray(rel_pos_indices)

    xg = x.reshape(GROUPS_TOTAL, GT, DIM)
    xT = np.ascontiguousarray(xg.transpose(0, 2, 1)).astype(BF16)
    xT = xT.reshape(GROUPS_TOTAL, 2, 128, GT)

    wq = w_qkv.copy()
    wq[:, :DIM] *= DH ** -0.5
    wqkv_h = np.ascontiguousarray(wq.reshape(2, 128, 768)).astype(BF16)
    wout_h = np.ascontiguousarray(w_out.reshape(2, 128, 256)).astype(BF16)

    bias = bias_table[rel_pos_indices]        # [60, 60, H]  (i, j, h)
    eb = np.exp(bias).transpose(1, 2, 0)      # [j, h, i]
    expb = np.zeros((GT, HEADS, GT), dtype=np.float32)
    for blk in range(2):
        expb[blk * N_TOK:(blk + 1) * N_TOK, :, blk * N_TOK:(blk + 1) * N_TOK] = eb
    expb_h = expb.astype(BF16)
    return xT, wqkv_h, wout_h, expb_h


def kernel(x, w_qkv, w_out, bias_table, rel_pos_indices):
    global LAST_RESULT
    from concourse.bass_utils import run_bass_kernel_spmd

    xT, wqkv_h, wout_h, expb_h = _host_prep(
        x, w_qkv, w_out, bias_table, rel_pos_indices
    )

    nc = _get_bass(NGROUPS)
    in_maps = []
    for c in range(N_CORES):
        in_maps.append({
            "xt": np.ascontiguousarray(xT[c * NGROUPS:(c + 1) * NGROUPS]),
            "wqkv": wqkv_h,
            "wout": wout_h,
            "expb": expb_h,
        })

    res = run_bass_kernel_spmd(
        nc, in_maps, core_ids=list(range(N_CORES)),
        trace=bool(int(os.environ.get("KERNEL_TRACE", "0"))),
    )
    LAST_RESULT = res

    out = np.concatenate([res.results[c]["out"] for c in range(N_CORES)], axis=0)
    out = out.reshape(16, 16, 16, WH, WW, DIM).astype(np.float32)
    return out


# revision 17
# speedup vs baseline: 1.9338x; 1.4928x over previous
"""Windowed multi-head attention (Swin-style) Bass kernel for Trainium2.

Full inputs -> shard over 8 NeuronCores (data-parallel over windows) -> full output.

Math per window w (n=60 tokens, d=256, h=8 heads, dh=32):
  qkv = x_w @ w_qkv ; sim = scale*q_h @ k_h^T + bias_h ; attn = softmax(sim)
  out_w = concat_h(attn @ v_h) @ w_out

Device formulation (per group of 2 windows, 120 token rows; groups processed
in quads for fat qkT streams):
  - host pre-transposes x -> xT [256, 120] (bf16), pre-scales w_q by dh^-0.5
  - qT, kT head-dim-major via matmul(lhsT=w_slice, rhs=xT), N=480 streams
    over 4 groups at once
  - simT for 4 heads in ONE matmul: lhsT = kT_chunk [128,120] dense,
    rhs = qT block-diagonal [128, 4*120] -> psum [120 (j), 480 (h,i)]
  - eT = exp(simT) * expbias  (expbias = exp(bias^T), 0 in cross-window
    blocks -> masks the 2-window batching AND the 4-head batching for free)
  - av per head: matmul(lhsT=eT_h, rhs=[v_h | ones]) -> [120, 33]: cols 0-31
    unnormalized attn@v token-major, col 32 = softmax denominator s_i
  - normalize: ONE broadcast multiply ao = av * (1/s)
  - PE-transpose ao -> aoT, proj: matmul(lhsT=aoT, rhs=w_out) -> [120, 256]
"""

import os
from contextlib import ExitStack

import numpy as np
import ml_dtypes

os.environ.setdefault("JAX_COMPILATION_CACHE_DIR", "/tmp/jaxcache")
os.environ.setdefault("JAX_PERSISTENT_CACHE_MIN_COMPILE_TIME_SECS", "2")

N_CORES = 8
WH, WW = 6, 10
N_TOK = WH * WW          # 60 tokens per window
DIM = 256
HEADS = 8
DH = 32
B_WINDOWS = 16 * 16 * 16  # 4096
GROUPS_TOTAL = B_WINDOWS // 2        # 2048 groups of 2 windows
NGROUPS = GROUPS_TOTAL // N_CORES    # 256 per core
GT = 2 * N_TOK           # 120 rows per group

BF16 = ml_dtypes.bfloat16

LAST_RESULT = None  # BassKernelResults from the most recent run (for test.py)

_CACHE = {}


def _build_bass(ngroups: int):
    import concourse.bass as bass
    import concourse.tile as tile
    from concourse import bacc, mybir
    from concourse.masks import make_identity

    fp32 = mybir.dt.float32
    bf16 = mybir.dt.bfloat16
    Copy = mybir.ActivationFunctionType.Copy
    Exp = mybir.ActivationFunctionType.Exp

    nc = bacc.Bacc("TRN2", debug=False, enable_asserts=False)

    xt_d = nc.dram_tensor("xt", [ngroups, 2, 128, GT], bf16, kind="ExternalInput").ap()
    wqkv_d = nc.dram_tensor("wqkv", [2, 128, 768], bf16, kind="ExternalInput").ap()
    wout_d = nc.dram_tensor("wout", [2, 128, 256], bf16, kind="ExternalInput").ap()
    expb_d = nc.dram_tensor("expb", [GT, HEADS, GT], bf16, kind="ExternalInput").ap()
    out_d = nc.dram_tensor("out", [ngroups, GT, 256], fp32, kind="ExternalOutput").ap()

    nquads = ngroups // 4

    with tile.TileContext(nc) as tc, ExitStack() as ctx:
        consts = ctx.enter_context(tc.tile_pool(name="consts", bufs=1))

        wqkv_sb = consts.tile([128, 2, 768], bf16)
        for kh in range(2):
            nc.gpsimd.dma_start(out=wqkv_sb[:, kh, :], in_=wqkv_d[kh])
        wout_sb = consts.tile([128, 2, 256], bf16)
        for kh in range(2):
            nc.gpsimd.dma_start(out=wout_sb[:, kh, :], in_=wout_d[kh])
        expb_sb = consts.tile([GT, HEADS, GT], bf16)
        nc.gpsimd.dma_start(out=expb_sb, in_=expb_d)
        ident = consts.tile([128, 128], bf16)
        make_identity(nc, ident)

        # block-diag qT staging, [chunk][quad parity]: [p, m(diag), gq, t];
        # zeroed once, only diagonal blocks rewritten each quad
        qbd = [
            [consts.tile([128, 4, 4, GT], bf16, name=f"qbd{c}{par}")
             for par in range(2)]
            for c in range(2)
        ]
        for c in range(2):
            for par in range(2):
                nc.vector.memset(qbd[c][par], 0.0)

        xpool = ctx.enter_context(tc.tile_pool(name="xp", bufs=3))
        ktpool = ctx.enter_context(tc.tile_pool(name="kt", bufs=2))
        etpool = ctx.enter_context(tc.tile_pool(name="et", bufs=4))
        vpool = ctx.enter_context(tc.tile_pool(name="vp", bufs=3))
        aopool = ctx.enter_context(tc.tile_pool(name="ao", bufs=3))
        aotpool = ctx.enter_context(tc.tile_pool(name="aot", bufs=3))
        rpool = ctx.enter_context(tc.tile_pool(name="rp", bufs=3))
        opool = ctx.enter_context(tc.tile_pool(name="op", bufs=4))

        pqk = ctx.enter_context(tc.tile_pool(name="pqk", bufs=2, space="PSUM"))
        pst = ctx.enter_context(tc.tile_pool(name="pst", bufs=2, space="PSUM"))
        pav = ctx.enter_context(tc.tile_pool(name="pav", bufs=2, space="PSUM"))
        pmisc = ctx.enter_context(tc.tile_pool(name="pmisc", bufs=2, space="PSUM"))

        for q in range(nquads):
            par = q % 2
            xt = xpool.tile([128, 2, 4, GT], bf16)  # [p, kh, gq, t]
            for kh in range(2):
                nc.sync.dma_start(
                    out=xt[:, kh, :, :],
                    in_=xt_d[4 * q:4 * q + 4, kh].rearrange("g p t -> p g t"),
                )

            # fat qkT matmuls over the quad: chunks 0,1 = q ; 2,3 = k
            kt = ktpool.tile([128, 2, 4, GT], bf16)
            for c in range(4):
                ps = pqk.tile([128, 4, GT], fp32, tag="ps")
                for kh in range(2):
                    nc.tensor.matmul(
                        ps.rearrange("p a b -> p (a b)"),
                        lhsT=wqkv_sb[:, kh, c * 128:(c + 1) * 128],
                        rhs=xt[:, kh, :, :].rearrange("p a b -> p (a b)"),
                        start=(kh == 0),
                        stop=(kh == 1),
                    )
                if c < 2:
                    # q chunk: scatter diagonal blocks (all 4 groups at once)
                    for m in range(4):
                        blk_out = qbd[c][par][m * DH:(m + 1) * DH, m, :, :]
                        blk_in = ps[m * DH:(m + 1) * DH, :, :]
                        if c == 0:
                            nc.vector.tensor_copy(out=blk_out, in_=blk_in)
                        else:
                            nc.scalar.activation(out=blk_out, in_=blk_in, func=Copy)
                else:
                    if c == 2:
                        nc.vector.tensor_copy(out=kt[:, 0, :, :], in_=ps)
                    else:
                        nc.scalar.activation(out=kt[:, 1, :, :], in_=ps, func=Copy)

            for gq in range(4):
                g = 4 * q + gq
                # v token-major with interleaved ones col: [120, h, 33]
                pv = pmisc.tile([GT, 256], fp32, tag="m")
                for kh in range(2):
                    nc.tensor.matmul(
                        pv,
                        lhsT=xt[:, kh, gq, :],
                        rhs=wqkv_sb[:, kh, 512:768],
                        start=(kh == 0),
                        stop=(kh == 1),
                    )
                v1 = vpool.tile([GT, HEADS, DH + 1], bf16)
                nc.gpsimd.memset(v1[:, :, DH:DH + 1], 1.0)
                nc.vector.tensor_copy(
                    out=v1[:, :, 0:DH],
                    in_=pv.rearrange("p (h d) -> p h d", h=HEADS),
                )

                # fat simT per chunk: [120 (j), 4*120 (h,i)]
                ets = []
                for c in range(2):
                    ps_sim = pst.tile([GT, 4, GT], fp32, tag="st")
                    nc.tensor.matmul(
                        ps_sim.rearrange("p a b -> p (a b)"),
                        lhsT=kt[:, c, gq, :],
                        rhs=qbd[c][par][:, :, gq, :],
                        start=True, stop=True,
                    )
                    et = etpool.tile([GT, 4, GT], bf16, tag="et")
                    nc.scalar.activation(out=et, in_=ps_sim, func=Exp)
                    nc.gpsimd.tensor_mul(et, et, expb_sb[:, 4 * c:4 * c + 4, :])
                    ets.append(et)

                # av: 8 small matmuls, inputs all ready -> pipeline on PE
                pav_t = pav.tile([GT, HEADS, DH + 1], fp32, tag="av")
                for h in range(8):
                    nc.tensor.matmul(
                        pav_t[:, h, :],
                        lhsT=ets[h // 4][:, h % 4, :],
                        rhs=v1[:, h, :],
                        start=True, stop=True,
                    )

                # normalize: r = 1/s ; ao = av * r (single broadcast multiply)
                r = rpool.tile([GT, HEADS, 1], fp32)
                nc.vector.reciprocal(out=r, in_=pav_t[:, :, DH:DH + 1])
                ao = aopool.tile([GT, 2, 128], bf16)
                aov = ao.rearrange("p a (b d) -> p (a b) d", d=DH)
                rb = bass.AP(tensor=r.tensor, offset=r.offset,
                             ap=[r.ap[0], r.ap[1], [0, DH]])
                nc.vector.tensor_mul(aov, pav_t[:, :, 0:DH], rb)

                # transpose ao -> aoT
                aoT = aotpool.tile([128, 2, GT], bf16)
                for kh in range(2):
                    pt = pmisc.tile([128, GT], bf16, tag="m")
                    nc.tensor.transpose(pt, ao[:, kh, :], ident[0:GT, 0:GT])
                    if kh == 0:
                        nc.vector.tensor_copy(out=aoT[:, kh, :], in_=pt)
                    else:
                        nc.scalar.activation(out=aoT[:, kh, :], in_=pt, func=Copy)

                # out projection
                po = pmisc.tile([GT, 256], fp32, tag="m")
                for kh in range(2):
                    nc.tensor.matmul(
                        po,
                        lhsT=aoT[:, kh, :],
                        rhs=wout_sb[:, kh, :],
                        start=(kh == 0),
                        stop=(kh == 1),
                    )
                osb = opool.tile([GT, 256], fp32)
                if gq % 2 == 0:
                    nc.scalar.activation(out=osb, in_=po, func=Copy)
                else:
                    nc.vector.tensor_copy(out=osb, in_=po)
                nc.sync.dma_start(out=out_d[g], in_=osb)

    nc.compile()
    return nc


def _get_bass(ngroups: int):
    if ngroups not in _CACHE:
        _CACHE[ngroups] = _build_bass(ngroups)
    return _CACHE[ngroups]


def _host_prep(x, w_qkv, w_out, bias_table, rel_pos_indices):
    x = np.asarray(x, dtype=np.float32)
    w_qkv = np.asarray(w_qkv, dtype=np.float32)
    w_out = np.asarray(w_out, dtype=np.float32)
    bias_table = np.asarray(bias_table, dtype=np.float32)
    rel_pos_indices = np.asarray(rel_pos_indices)

    xg = x.reshape(GROUPS_TOTAL, GT, DIM)
    xT = np.ascontiguousarray(xg.transpose(0, 2, 1)).astype(BF16)
    xT = xT.reshape(GROUPS_TOTAL, 2, 128, GT)

    wq = w_qkv.copy()
    wq[:, :DIM] *= DH ** -0.5
    wqkv_h = np.ascontiguousarray(wq.reshape(2, 128, 768)).astype(BF16)
    wout_h = np.ascontiguousarray(w_out.reshape(2, 128, 256)).astype(BF16)

    bias = bias_table[rel_pos_indices]        # [60, 60, H]  (i, j, h)
    eb = np.exp(bias).transpose(1, 2, 0)      # [j, h, i]
    expb = np.zeros((GT, HEADS, GT), dtype=np.float32)
    for blk in range(2):
        expb[blk * N_TOK:(blk + 1) * N_TOK, :, blk * N_TOK:(blk + 1) * N_TOK] = eb
    expb_h = expb.astype(BF16)
    return xT, wqkv_h, wout_h, expb_h


def kernel(x, w_qkv, w_out, bias_table, rel_pos_indices):
    global LAST_RESULT
    from concourse.bass_utils import run_bass_kernel_spmd

    xT, wqkv_h, wout_h, expb_h = _host_prep(
        x, w_qkv, w_out, bias_table, rel_pos_indices
    )

    nc = _get_bass(NGROUPS)
    in_maps = []
    for c in range(N_CORES):
        in_maps.append({
            "xt": np.ascontiguousarray(xT[c * NGROUPS:(c + 1) * NGROUPS]),
            "wqkv": wqkv_h,
            "wout": wout_h,
            "expb": expb_h,
        })

    res = run_bass_kernel_spmd(
        nc, in_maps, core_ids=list(range(N_CORES)),
        trace=bool(int(os.environ.get("KERNEL_TRACE", "0"))),
    )
    LAST_RESULT = res

    out = np.concatenate([res.results[c]["out"] for c in range(N_CORES)], axis=0)
    out = out.reshape(16, 16, 16, WH, WW, DIM).astype(np.float32)
    return out
